# revision 1
# baseline (speedup 1.0000x reference)
"""Trainium2 Bass kernel for nn_CausalSelfAttention_35931696398729.

Sharding: 8 cores = (batch b in {0,1}) x (kv-head n in {0..3}).
Each core computes its 4 query heads' causal GQA attention for its batch
plus the partial c_proj (rows of Wo for its heads); the host sums the 4
partials per batch.  No device collectives.

Layouts are "transposed" throughout: qT/kT (d on partitions, t free) so
scores come out as ST (keys on partitions, queries free) and PV consumes
exp(ST) directly;  c_proj consumes the attention output OT (d, t) as the
stationary operand with no transposes anywhere except V (16 PE-transposes).

QK RMSNorm is folded in without normalizing q/k tensors elementwise:
 - q-side factor r_q(t)/sqrt(HD) multiplies qT columns (query temperature)
 - k-side factor r_k(s) rides the Exp activation's per-partition scale
 - gamma_q*gamma_k multiplies kT rows (per-partition)
 - softmax runs without max-subtraction (|scores| <= sqrt(HD) after norm)
 - 1/rowsum is applied to OT columns after PV.
"""

import os
import sys

sys.path.insert(0, "/opt/trn_rl_repo")

import numpy as np

import concourse.bacc as bacc
import concourse.mybir as mybir
import concourse.tile as tile
from concourse import bass_utils
from concourse.masks import make_identity

B, T, D = 2, 2048, 2048
NH, NKV, HD = 16, 4, 128
G = NH // NKV  # query heads per core
EPS = 1e-6
THETA = 10000.0
N_CORES = 8
P = 128
TC = 512            # q-chunk for attention / c_proj column chunk
NTC = T // TC       # 4
TC1 = 256           # t-chunk for phase-1 projections
NTC1 = T // TC1     # 8
NKT = D // P        # 16 contraction chunks
NTB = T // P        # 16 t-blocks

MM_MODE = os.environ.get("KERNEL_MM_DT", "float32r")
# x activations in bf16 (halves the dominant input DMA); weights stay MM_DT
X_BF16 = os.environ.get("KERNEL_X_BF16", "0") == "1" and MM_MODE == "float32r"
F32 = mybir.dt.float32
MM_DT = {"float32": F32, "float32r": mybir.dt.float32r,
         "bfloat16": mybir.dt.bfloat16}[MM_MODE]
# storage dtype for matmul operands (fp32r operands must be *produced* as
# fp32r for the BIR verifier, so operand tiles use the matmul dtype).
ST_DT = MM_DT
NP_ST = np.dtype("bfloat16") if MM_MODE == "bfloat16" else np.float32


def _mm(ap):
    return ap


def build_program():
    nc = bacc.Bacc("TRN2", target_bir_lowering=False, debug=False,
                   enable_asserts=False, num_devices=N_CORES)

    x_dt = mybir.dt.bfloat16 if X_BF16 else ST_DT
    xT = nc.dram_tensor("xT", (D, T), x_dt, kind="ExternalInput").ap()
    wq = nc.dram_tensor("wq", (D, G * HD), x_dt, kind="ExternalInput").ap()
    wk = nc.dram_tensor("wk", (D, HD), x_dt, kind="ExternalInput").ap()
    wv = nc.dram_tensor("wv", (D, HD), x_dt, kind="ExternalInput").ap()
    wo = nc.dram_tensor("wo", (G * HD, D), ST_DT, kind="ExternalInput").ap()
    cosT = nc.dram_tensor("cosT", (P, T), F32, kind="ExternalInput").ap()
    sinT = nc.dram_tensor("sinT", (P, T), F32, kind="ExternalInput").ap()
    gamma2 = nc.dram_tensor("gamma2", (P, 1), F32, kind="ExternalInput").ap()
    y = nc.dram_tensor("y", (T, D), F32, kind="ExternalOutput").ap()

    with tile.TileContext(nc) as tc, \
         nc.allow_low_precision(reason="fp32r/bf16 matmul operand tiles"):
        with tc.tile_pool(name="persist", bufs=1) as persist, \
             tc.tile_pool(name="stri2", bufs=2) as stri2:
            cos_sb = persist.tile([P, T], F32)
            sin_sb = persist.tile([P, T], F32)
            g2_sb = persist.tile([P, 1], F32)
            ident_f32 = persist.tile([P, P], F32)
            make_identity(nc, ident_f32)
            ident = persist.tile([P, P], ST_DT)
            nc.vector.tensor_copy(out=ident, in_=ident_f32)
            ones_f32 = persist.tile([P, P], F32)
            nc.vector.memset(ones_f32, 1.0)
            ones_col = persist.tile([P, 1], ST_DT)
            nc.vector.tensor_copy(out=ones_col, in_=ones_f32[:, 0:1])
            ones_st = persist.tile([P, P], ST_DT)
            nc.vector.tensor_copy(out=ones_st, in_=ones_f32)
            eps_k = persist.tile([P, 1], F32)
            nc.vector.memset(eps_k, EPS)
            eps_q = persist.tile([1, 1], F32)
            nc.vector.memset(eps_q, HD * EPS)

            q_sb = [persist.tile([P, T], ST_DT, tag=f"q_sb{h}", name=f"q_sb{h}")
                    for h in range(G)]
            kT_sb = persist.tile([P, T], ST_DT)
            v_sb = persist.tile([P, NTB, P], ST_DT)
            # q-norm stripes: heads 0/1 on partitions 0/32 of stripe_a,
            # heads 2/3 on partitions 0/32 of stripe_b (matmul base
            # partition must be one of 0/32/64).
            stripe_a = persist.tile([P, T], ST_DT)
            stripe_b = persist.tile([P, T], ST_DT)
            rk_tiles = persist.tile([P, NTB], F32)

            def rq_row(h):
                return (stripe_a if h < 2 else stripe_b), 32 * (h % 2)

            # PE warm-up: the first ~7us are DMA-bound with PE idle and the
            # HAM clock gate cold (half rate).  Tiny dummy matmuls on the
            # identity tile (ready immediately) keep PE "busy" through the
            # ramp window so the first real projection matmuls run warm.
            # The pool is released before phase 1 so its PSUM bank is reused.
            with tc.tile_pool(name="warm", bufs=1, space="PSUM") as ps_w:
                warm_ps = ps_w.tile([1, P], F32)
                for _ in range(28):
                    nc.tensor.matmul(warm_ps, ident_f32[:, 0:1],
                                     ident_f32, start=True, stop=True)

            # ---------------- Phase 1: projections + RoPE + norms -----------
            with tc.tile_pool(name="weights", bufs=1) as wpool, \
                 tc.tile_pool(name="xts", bufs=2) as xpool, \
                 tc.tile_pool(name="p1tmp", bufs=3) as tmpool, \
                 tc.tile_pool(name="p1ps", bufs=4, space="PSUM") as ps_a, \
                 tc.tile_pool(name="p1psv", bufs=2, space="PSUM") as ps_v, \
                 tc.tile_pool(name="p1str", bufs=1, space="PSUM") as ps_s:
                wq_sb = wpool.tile([P, NKT, G * HD], x_dt)
                wk_sb = wpool.tile([P, NKT, HD], x_dt)
                wv_sb = wpool.tile([P, NKT, HD], x_dt)
                nc.sync.dma_start(out=wk_sb,
                                  in_=wk.rearrange("(kt p) m -> p kt m", p=P))

                def rope_from_psum(dst, ps, sl, sq_dt=F32):
                    """dst[:, sl] = rope(ps); returns tile holding square."""
                    tmp = tmpool.tile([P, TC1], F32, tag="ropetmp",
                                      name="ropetmp")
                    # tmp = swap(ps) * sinT
                    nc.vector.tensor_mul(out=tmp[0:64, :], in0=ps[64:128, :],
                                         in1=sin_sb[0:64, sl])
                    nc.vector.tensor_mul(out=tmp[64:128, :], in0=ps[0:64, :],
                                         in1=sin_sb[64:128, sl])
                    # dst = ps * cosT + tmp
                    tmp2 = tmpool.tile([P, TC1], F32, tag="ropetmp2",
                                       name="ropetmp2")
                    nc.vector.tensor_mul(out=tmp2, in0=ps, in1=cos_sb[:, sl])
                    nc.vector.tensor_add(out=dst[:, sl], in0=tmp2, in1=tmp)
                    # square for the norm (ACT is idle here); for the q
                    # path write it as the matmul dtype so the ssq matmul
                    # can run at fp32r speed (fp32 matmul is 4 cyc/row)
                    sqt = tmpool.tile([P, TC1], sq_dt, tag="ropesq",
                                      name="ropesq")
                    nc.scalar.square(out=sqt, in_=dst[:, sl])
                    return sqt

                for tc_i in range(NTC1):
                    sl = slice(tc_i * TC1, (tc_i + 1) * TC1)
                    xts = xpool.tile([P, NKT, TC1], x_dt, tag="xts",
                                     name="xts")
                    xre = xT[:, sl].rearrange("(kt p) m -> p kt m", p=P)
                    if tc_i == 0:
                        # stage startup loads so PE starts ASAP: k weights +
                        # first x chunks first, then tables, then q/v weights
                        for kg in range(2):
                            nc.sync.dma_start(
                                out=xts[:, 4 * kg:4 * (kg + 1), :],
                                in_=xre[:, 4 * kg:4 * (kg + 1), :])
                        nc.sync.dma_start(out=cos_sb, in_=cosT)
                        nc.sync.dma_start(out=sin_sb, in_=sinT)
                        for kg in range(2, 4):
                            nc.sync.dma_start(
                                out=xts[:, 4 * kg:4 * (kg + 1), :],
                                in_=xre[:, 4 * kg:4 * (kg + 1), :])
                        nc.sync.dma_start(out=g2_sb, in_=gamma2)
                        wqre = wq.rearrange("(kt p) m -> p kt m", p=P)
                        for h in range(G):
                            nc.sync.dma_start(
                                out=wq_sb[:, :, h * HD:(h + 1) * HD],
                                in_=wqre[:, :, h * HD:(h + 1) * HD])
                        nc.sync.dma_start(
                            out=wv_sb,
                            in_=wv.rearrange("(kt p) m -> p kt m", p=P))
                    else:
                        for kg in range(4):
                            nc.sync.dma_start(
                                out=xts[:, 4 * kg:4 * (kg + 1), :],
                                in_=xre[:, 4 * kg:4 * (kg + 1), :])

                    # ---- K ----
                    ps = ps_a.tile([P, TC1], F32, tag="proj", name="ps_k")
                    for kt in range(NKT):
                        nc.tensor.matmul(ps, _mm(wk_sb[:, kt, :]),
                                         _mm(xts[:, kt, :]),
                                         start=(kt == 0), stop=(kt == NKT - 1))
                    sq = rope_from_psum(kT_sb, ps, sl)
                    # rk column tiles: 1/sqrt(colsum(sq)/HD + eps) per key block
                    for i in range(TC1 // P):
                        kb = tc_i * (TC1 // P) + i
                        ssqc = ps_s.tile([P, 1], F32, tag="ssqc",
                                         name="ssqc")
                        nc.tensor.matmul(ssqc,
                                         sq[:, i * P:(i + 1) * P],
                                         ones_f32[:, 0:1],
                                         start=True, stop=True)
                        nc.scalar.activation(
                            out=rk_tiles[:, kb:kb + 1], in_=ssqc,
                            func=mybir.ActivationFunctionType.Sqrt,
                            bias=eps_k[:], scale=float(1.0 / HD))
                        nc.vector.reciprocal(out=rk_tiles[:, kb:kb + 1],
                                             in_=rk_tiles[:, kb:kb + 1])
                    # gamma2 applied after the norm-square
                    nc.vector.tensor_scalar_mul(out=kT_sb[:, sl],
                                                in0=kT_sb[:, sl], scalar1=g2_sb)

                    # ---- Q heads ----
                    for h in range(G):
                        ps = ps_a.tile([P, TC1], F32, tag="proj",
                                       name="ps_q")
                        for kt in range(NKT):
                            nc.tensor.matmul(
                                ps, _mm(wq_sb[:, kt, h * HD:(h + 1) * HD]),
                                _mm(xts[:, kt, :]),
                                start=(kt == 0), stop=(kt == NKT - 1))
                        sq = rope_from_psum(q_sb[h], ps, sl,
                                            sq_dt=ST_DT)
                        ssq = ps_s.tile([1, TC1], F32, tag="ssq",
                                        name="ssq_q")
                        nc.tensor.matmul(ssq, _mm(ones_col), _mm(sq),
                                         start=True, stop=True)
                        # rq/sqrt(HD) = 1/sqrt(ssq + HD*eps)
                        sq_s = stri2.tile([1, TC1], F32, tag="sqs",
                                          name="sq_sq")
                        nc.scalar.activation(
                            out=sq_s, in_=ssq,
                            func=mybir.ActivationFunctionType.Sqrt,
                            bias=eps_q[:], scale=1.0)
                        s_t, r0 = rq_row(h)
                        nc.vector.reciprocal(
                            out=s_t[r0:r0 + 1, sl], in_=sq_s)

                    # ---- V (computed transposed, then PE-transposed) ----
                    ps = ps_a.tile([P, TC1], F32, tag="proj", name="ps_vp")
                    for kt in range(NKT):
                        nc.tensor.matmul(ps, _mm(wv_sb[:, kt, :]),
                                         _mm(xts[:, kt, :]),
                                         start=(kt == 0), stop=(kt == NKT - 1))
                    vt_sb = tmpool.tile([P, TC1], ST_DT, tag="vt", name="vt")
                    nc.vector.tensor_copy(out=vt_sb, in_=ps)
                    for i in range(TC1 // P):
                        pst = ps_v.tile([P, P], ST_DT, tag="vtr", name="pst")
                        nc.tensor.transpose(pst, vt_sb[:, i * P:(i + 1) * P],
                                            ident)
                        nc.vector.tensor_copy(
                            out=v_sb[:, tc_i * (TC1 // P) + i, :], in_=pst)

            # ---------------- Phase 2: attention ---------------------------
            with tc.tile_pool(name="wo", bufs=1) as wopool, \
                 tc.tile_pool(name="attn", bufs=2) as apool, \
                 tc.tile_pool(name="psb", bufs=6) as ppool, \
                 tc.tile_pool(name="otn", bufs=1) as otpool:
                wo_sb = wopool.tile([P, G, D], ST_DT)
                nc.sync.dma_start(out=wo_sb,
                                  in_=wo.rearrange("(h p) m -> p h m", p=P))
                otn_sb = [otpool.tile([P, T], ST_DT, tag=f"otn{h}",
                                      name=f"otn{h}")
                          for h in range(G)]

                with tc.tile_pool(name="p2st", bufs=3, space="PSUM") as ps_st, \
                     tc.tile_pool(name="p2ot", bufs=2, space="PSUM") as ps_ot, \
                     tc.tile_pool(name="p2rs", bufs=1, space="PSUM") as ps_rs, \
                     tc.tile_pool(name="p3ya", bufs=1, space="PSUM") as ps_ya, \
                     tc.tile_pool(name="p3yb", bufs=1, space="PSUM") as ps_yb, \
                     tc.tile_pool(name="ysb", bufs=6) as ypool:
                    def temper_chunk(tc_i):
                        # q *= r_q/sqrt(HD) for one 512-col chunk, all heads
                        tsl = slice(tc_i * TC, (tc_i + 1) * TC)
                        for h in range(G):
                            s_t, r0 = rq_row(h)
                            rb_ps = ps_ya.tile([P, TC], F32, tag="ya",
                                               name="rb_ps")
                            nc.tensor.matmul(
                                rb_ps, _mm(ones_st[r0:r0 + 1, :]),
                                _mm(s_t[r0:r0 + 1, tsl]),
                                start=True, stop=True)
                            nc.vector.tensor_mul(out=q_sb[h][:, tsl],
                                                 in0=q_sb[h][:, tsl],
                                                 in1=rb_ps)

                    # temper chunks staggered one qc ahead of use, so the
                    # in-order engine queues never wait on late phase-1 tiles
                    temper_chunk(0)
                    temper_chunk(1)
                    for qc in range(NTC):
                        qsl = slice(qc * TC, (qc + 1) * TC)
                        nkb = 4 * (qc + 1)
                        if qc + 2 < NTC:
                            pass  # tempered below, after this qc's attention
                        recips = []
                        ots = []
                        for h in range(G):
                            ot_ps = ps_ot.tile([P, TC], F32, tag="ot",
                                               name="ot_ps")
                            rs_ps = ps_rs.tile([1, TC], F32, tag="rs",
                                               name="rs_ps")
                            for kb in range(nkb):
                                r = kb - 4 * qc  # >=0 on diagonal blocks
                                c0 = max(r, 0) * P  # first valid q column
                                st_ps = ps_st.tile([P, TC], F32, tag="st",
                                                   name="st_ps")
                                nc.tensor.matmul(
                                    st_ps[:, c0:],
                                    _mm(kT_sb[:, kb * P:(kb + 1) * P]),
                                    _mm(q_sb[h][:, qc * TC + c0:
                                                (qc + 1) * TC]),
                                    start=True, stop=True)
                                p_sb = ppool.tile([P, TC], ST_DT, tag="p",
                                                  name="p_sb")
                                nc.scalar.activation(
                                    out=p_sb[:, c0:], in_=st_ps[:, c0:],
                                    func=mybir.ActivationFunctionType.Exp,
                                    scale=rk_tiles[:, kb:kb + 1])
                                if r >= 0:
                                    # causal mask on the diagonal strip only
                                    # (PV/rowsum read cols >= c0): keep iff
                                    # col - p >= 0 within the strip
                                    nc.gpsimd.affine_select(
                                        out=p_sb[:, c0:c0 + P],
                                        in_=p_sb[:, c0:c0 + P],
                                        pattern=[[1, P]],
                                        compare_op=mybir.AluOpType.is_ge,
                                        fill=0.0,
                                        base=0,
                                        channel_multiplier=-1)
                                nc.tensor.matmul(
                                    ot_ps[:, c0:], _mm(v_sb[:, kb, :]),
                                    _mm(p_sb[:, c0:]), start=(kb == 0),
                                    stop=(kb == nkb - 1))
                                nc.tensor.matmul(
                                    rs_ps[:, c0:], _mm(ones_col),
                                    _mm(p_sb[:, c0:]), start=(kb == 0),
                                    stop=(kb == nkb - 1))
                            recip = stri2.tile([1, TC], ST_DT, tag="recip",
                                               name="recip", bufs=4)
                            nc.vector.reciprocal(out=recip, in_=rs_ps)
                            recips.append(recip)
                            ots.append(ot_ps)
                        # deferred normalization: overlaps next qc attention
                        for h in range(G):
                            recipB = apool.tile([P, TC], ST_DT, tag="recipB",
                                                name="recipB", bufs=4)
                            nc.gpsimd.partition_broadcast(recipB, recips[h])
                            nc.vector.tensor_mul(out=otn_sb[h][:, qsl],
                                                 in0=ots[h], in1=recipB)

                        # c_proj for this qc's four t-blocks, interleaved so
                        # PE keeps busy while the next qc's attention starts.
                        for tb in range(4 * qc, 4 * qc + 4):
                            for jg in (0, 2):
                                ya = ps_ya.tile([P, TC], F32, tag="ya",
                                                name="ya")
                                yb = ps_yb.tile([P, TC], F32, tag="yb",
                                                name="yb")
                                for h in range(G):
                                    lhs = otn_sb[h][:, tb * P:(tb + 1) * P]
                                    nc.tensor.matmul(
                                        ya, _mm(lhs),
                                        _mm(wo_sb[:, h, jg * TC:(jg + 1) * TC]),
                                        start=(h == 0), stop=(h == G - 1))
                                    nc.tensor.matmul(
                                        yb, _mm(lhs),
                                        _mm(wo_sb[:, h,
                                                  (jg + 1) * TC:(jg + 2) * TC]),
                                        start=(h == 0), stop=(h == G - 1))
                                for j, yp in ((jg, ya), (jg + 1, yb)):
                                    y_sb = ypool.tile([P, TC], F32,
                                                      tag="y_sb", name="y_sb")
                                    nc.vector.tensor_copy(out=y_sb, in_=yp)
                                    nc.sync.dma_start(
                                        out=y[tb * P:(tb + 1) * P,
                                              j * TC:(j + 1) * TC],
                                        in_=y_sb)

                        if qc + 2 < NTC:
                            temper_chunk(qc + 2)

    nc.compile()
    return nc


_NC_CACHE = None


def _get_program():
    global _NC_CACHE
    if _NC_CACHE is None:
        _NC_CACHE = build_program()
    return _NC_CACHE


def _make_tables(pos):
    half = HD // 2
    inv_freq = 1.0 / (THETA ** (np.arange(half, dtype=np.float64) / half))
    ang = (pos + np.arange(T, dtype=np.float64))[None, :] * inv_freq[:, None]
    cos = np.cos(ang).astype(np.float32)
    sin = np.sin(ang).astype(np.float32)
    cosT = np.ascontiguousarray(np.concatenate([cos, cos], axis=0))
    sinT = np.ascontiguousarray(np.concatenate([-sin, sin], axis=0))
    return cosT, sinT


def kernel(x, Wq, Wk, Wv, Wo, q_gamma, k_gamma, pos):
    x = np.asarray(x, dtype=np.float32)
    Wq = np.asarray(Wq, dtype=np.float32)
    Wk = np.asarray(Wk, dtype=np.float32)
    Wv = np.asarray(Wv, dtype=np.float32)
    Wo = np.asarray(Wo, dtype=np.float32)
    q_gamma = np.asarray(q_gamma, dtype=np.float32)
    k_gamma = np.asarray(k_gamma, dtype=np.float32)
    pos = int(np.asarray(pos))

    cosT, sinT = _make_tables(pos)
    gamma2 = np.ascontiguousarray((q_gamma * k_gamma).reshape(P, 1)
                                  .astype(np.float32))

    def st(a):
        return np.ascontiguousarray(a.astype(NP_ST))

    in_maps = []
    for c in range(N_CORES):
        b, n = divmod(c, NKV)
        def xt_(a):
            return np.ascontiguousarray(
                a.astype(np.dtype("bfloat16") if X_BF16 else NP_ST))
        xst = xt_(x[b].T)
        in_maps.append({
            "xT": xst,
            "wq": xt_(Wq[:, n * G * HD:(n + 1) * G * HD]),
            "wk": xt_(Wk[:, n * HD:(n + 1) * HD]),
            "wv": xt_(Wv[:, n * HD:(n + 1) * HD]),
            "wo": st(Wo[n * G * HD:(n + 1) * G * HD, :]),
            "cosT": cosT,
            "sinT": sinT,
            "gamma2": gamma2,
        })

    nc = _get_program()
    res = bass_utils.run_bass_kernel_spmd(nc, in_maps,
                                          core_ids=list(range(N_CORES)))
    out = np.zeros((B, T, D), dtype=np.float32)
    for c in range(N_CORES):
        b = c // NKV
        out[b] += res.results[c]["y"]
    return out


if __name__ == "__main__":
    build_program()
    print("program built OK")



# revision 14
# speedup vs baseline: 1.0615x; 1.0615x over previous
"""Trainium2 Bass kernel for nn_CausalSelfAttention_35931696398729.

Sharding: 8 cores = (batch b in {0,1}) x (kv-head n in {0..3}).
Each core computes its 4 query heads' causal GQA attention for its batch
plus the partial c_proj (rows of Wo for its heads); the host sums the 4
partials per batch.  No device collectives.

All matmul operands are fp16 (1 cyc/row in the cost model, like bf16,
with 8x lower quantization error).  PSUM stays f32.

Key structure:
 - qT/kT (d on partitions, t free) so scores come out as ST (keys on
   partitions, queries free) and PV consumes exp(ST) directly.
 - V is projected directly in [t, d] layout (x block as the stationary
   operand) -- no PE transposes.
 - QK RMSNorm: squares are taken from the PRE-RoPE psum (rotation
   preserves column norms).  q-side factor rq(t)/sqrt(HD) is multiplied
   into q during phase 1 (Pool partition_broadcast + DVE mul); k-side
   factor rk(s) rides the Exp activation's per-partition scale;
   gamma_q*gamma_k is folded into the K RoPE tables on the host.
 - softmax runs without max-subtraction but with a constant -2 bias in
   the exponent (softmax-invariant) so exp stays in fp16 range.
 - rowsum: P_acc += p on DVE (scalar_tensor_tensor, 4x mode), then one
   [1,TC] matmul per (head, q-chunk); 1/rowsum applied to OT columns.
 - c_proj for q-chunk qc-1 is interleaved into attention of qc at key-
   block granularity; y copies run on the Pool engine; y is fp16.
"""

import os
import sys

sys.path.insert(0, "/opt/trn_rl_repo")

import numpy as np

import concourse.bacc as bacc
import concourse.mybir as mybir
import concourse.tile as tile
from concourse import bass_utils

B, T, D = 2, 2048, 2048
NH, NKV, HD = 16, 4, 128
G = NH // NKV  # query heads per core
EPS = 1e-6
THETA = 10000.0
N_CORES = 8
P = 128
TC = 512            # q-chunk for attention / c_proj column chunk
NTC = T // TC       # 4
TC1 = 256           # t-chunk for phase-1 projections
NTC1 = T // TC1     # 8
NKT = D // P        # 16 contraction chunks
NTB = T // P        # 16 t-blocks
EXP_BIAS = -2.0     # constant exponent shift (softmax invariant)

F32 = mybir.dt.float32
DT = mybir.dt.float16
NP_DT = np.float16


def build_program():
    nc = bacc.Bacc("TRN2", target_bir_lowering=False, debug=False,
                   enable_asserts=False, num_devices=N_CORES)

    xT = nc.dram_tensor("xT", (D, T), DT, kind="ExternalInput").ap()
    wq = nc.dram_tensor("wq", (D, G * HD), DT, kind="ExternalInput").ap()
    wk = nc.dram_tensor("wk", (D, HD), DT, kind="ExternalInput").ap()
    wv = nc.dram_tensor("wv", (D, HD), DT, kind="ExternalInput").ap()
    wo = nc.dram_tensor("wo", (G * HD, D), DT, kind="ExternalInput").ap()
    cosq = nc.dram_tensor("cosq", (P, T), DT, kind="ExternalInput").ap()
    sinq = nc.dram_tensor("sinq", (P, T), DT, kind="ExternalInput").ap()
    cosk = nc.dram_tensor("cosk", (P, T), DT, kind="ExternalInput").ap()
    sink = nc.dram_tensor("sink", (P, T), DT, kind="ExternalInput").ap()
    y = nc.dram_tensor("y", (T, D), DT, kind="ExternalOutput").ap()

    AF = mybir.ActivationFunctionType
    ALU = mybir.AluOpType

    with tile.TileContext(nc) as tc, \
         nc.allow_low_precision(reason="fp16 matmul/softmax pipeline"):
        with tc.tile_pool(name="persist", bufs=1) as persist, \
             tc.tile_pool(name="stri", bufs=4) as stri:
            cosq_sb = persist.tile([P, T], DT)
            sinq_sb = persist.tile([P, T], DT)
            cosk_sb = persist.tile([P, T], DT)
            sink_sb = persist.tile([P, T], DT)
            ones_col = persist.tile([P, 1], DT)
            nc.vector.memset(ones_col, 1.0)
            warm_src = persist.tile([P, P], DT)
            nc.vector.memset(warm_src, 0.0)
            eps_k = persist.tile([P, 1], F32)
            nc.vector.memset(eps_k, EPS)
            eps_q = persist.tile([1, 1], F32)
            nc.vector.memset(eps_q, HD * EPS)
            expb = persist.tile([P, 1], F32)
            nc.vector.memset(expb, EXP_BIAS)

            q_sb = [persist.tile([P, T], DT, tag=f"q_sb{h}", name=f"q_sb{h}")
                    for h in range(G)]
            kT_sb = persist.tile([P, T], DT)
            v_sb = persist.tile([P, NTB, P], DT)
            rk_tiles = persist.tile([P, NTB], F32)
            wo_sb = persist.tile([P, G, D], DT)
            otn_sb = [persist.tile([P, T], DT, tag=f"otn{h}", name=f"otn{h}")
                      for h in range(G)]

            # PE warm-up: keep PE busy through the cold-clock ramp window
            # while the first x chunks stream in.
            with tc.tile_pool(name="warm", bufs=1, space="PSUM") as ps_w:
                warm_ps = ps_w.tile([1, P], F32)
                for _ in range(24):
                    nc.tensor.matmul(warm_ps, ones_col, warm_src,
                                     start=True, stop=True)

            # ---------------- Phase 1: projections + RoPE + norms -----------
            with tc.tile_pool(name="weights", bufs=1) as wpool, \
                 tc.tile_pool(name="xts", bufs=2) as xpool, \
                 tc.tile_pool(name="p1tmp", bufs=3) as tmpool, \
                 tc.tile_pool(name="p1q", bufs=5) as qpool, \
                 tc.tile_pool(name="p1ps", bufs=3, space="PSUM") as ps_a, \
                 tc.tile_pool(name="p1psv", bufs=2, space="PSUM") as ps_v, \
                 tc.tile_pool(name="p1sc", bufs=1, space="PSUM") as ps_sc, \
                 tc.tile_pool(name="p1sq", bufs=2, space="PSUM") as ps_sq:
                wq_sb = wpool.tile([P, NKT, G * HD], DT)
                wk_sb = wpool.tile([P, NKT, HD], DT)
                wv_sb = wpool.tile([P, NKT, HD], DT)
                nc.sync.dma_start(out=wk_sb,
                                  in_=wk.rearrange("(kt p) m -> p kt m", p=P))

                def swap_copy(psb, tag):
                    # halves-swapped copy (single-input ops may cross
                    # partition bases; two-input SB+SB ops may not)
                    psb_sw = tmpool.tile([P, TC1], DT, tag=tag, name=tag)
                    nc.vector.tensor_copy(out=psb_sw[0:64, :],
                                          in_=psb[64:128, :])
                    nc.vector.tensor_copy(out=psb_sw[64:128, :],
                                          in_=psb[0:64, :])
                    return psb_sw

                def rope(psb, psb_sw, cos_t, sin_t, dst):
                    # dst = psb * cos + swap(psb) * sin   (all fp16 SBUF,
                    # partition-aligned; sin table carries the sign fold)
                    tmp = tmpool.tile([P, TC1], DT, tag="ropetmp",
                                      name="ropetmp")
                    nc.vector.tensor_mul(out=tmp, in0=psb_sw, in1=sin_t)
                    tmp2 = tmpool.tile([P, TC1], DT, tag="ropetmp2",
                                       name="ropetmp2")
                    nc.vector.tensor_mul(out=tmp2, in0=psb, in1=cos_t)
                    nc.vector.tensor_add(out=dst, in0=tmp2, in1=tmp)

                for tc_i in range(NTC1):
                    sl = slice(tc_i * TC1, (tc_i + 1) * TC1)
                    xts = xpool.tile([P, NKT, TC1], DT, tag="xts", name="xts")
                    xre = xT[:, sl].rearrange("(kt p) m -> p kt m", p=P)
                    if tc_i == 0:
                        # stage startup loads: k weights + first x chunks
                        # first, then tables, then q/v weights, wo last
                        for kg in range(2):
                            nc.sync.dma_start(
                                out=xts[:, 4 * kg:4 * (kg + 1), :],
                                in_=xre[:, 4 * kg:4 * (kg + 1), :])
                        nc.sync.dma_start(out=cosk_sb, in_=cosk)
                        nc.sync.dma_start(out=sink_sb, in_=sink)
                        for kg in range(2, 4):
                            nc.sync.dma_start(
                                out=xts[:, 4 * kg:4 * (kg + 1), :],
                                in_=xre[:, 4 * kg:4 * (kg + 1), :])
                        nc.sync.dma_start(out=cosq_sb, in_=cosq)
                        nc.sync.dma_start(out=sinq_sb, in_=sinq)
                        wqre = wq.rearrange("(kt p) m -> p kt m", p=P)
                        for h in range(G):
                            nc.sync.dma_start(
                                out=wq_sb[:, :, h * HD:(h + 1) * HD],
                                in_=wqre[:, :, h * HD:(h + 1) * HD])
                        nc.sync.dma_start(
                            out=wv_sb,
                            in_=wv.rearrange("(kt p) m -> p kt m", p=P))
                        nc.sync.dma_start(
                            out=wo_sb,
                            in_=wo.rearrange("(h p) m -> p h m", p=P))
                    else:
                        for kg in range(4):
                            nc.sync.dma_start(
                                out=xts[:, 4 * kg:4 * (kg + 1), :],
                                in_=xre[:, 4 * kg:4 * (kg + 1), :])

                    # ---- PE: projections (K, Q heads, V) -------------------
                    ps_k = ps_a.tile([P, TC1], F32, tag="proj", name="ps_k")
                    for kt in range(NKT):
                        nc.tensor.matmul(ps_k, wk_sb[:, kt, :],
                                         xts[:, kt, :],
                                         start=(kt == 0), stop=(kt == NKT - 1))
                    # Act: psum -> fp16 SBUF copy + square (pre-RoPE norm)
                    psb_k = tmpool.tile([P, TC1], DT, tag="psb", name="psb_k")
                    nc.scalar.copy(out=psb_k, in_=ps_k)
                    psw_k = swap_copy(psb_k, "psw")
                    sq_k = tmpool.tile([P, TC1], DT, tag="sq", name="sq_k")
                    nc.scalar.square(out=sq_k, in_=psb_k)
                    rope(psb_k, psw_k, cosk_sb[:, sl], sink_sb[:, sl],
                         kT_sb[:, sl])

                    q_ps = []
                    for h in range(G):
                        ps_q = ps_a.tile([P, TC1], F32, tag="proj",
                                         name="ps_q")
                        for kt in range(NKT):
                            nc.tensor.matmul(
                                ps_q, wq_sb[:, kt, h * HD:(h + 1) * HD],
                                xts[:, kt, :],
                                start=(kt == 0), stop=(kt == NKT - 1))
                        psb_q = qpool.tile([P, TC1], DT, tag="psbq",
                                           name="psb_q")
                        nc.scalar.copy(out=psb_q, in_=ps_q)
                        psw_q = swap_copy(psb_q, "pswq")
                        sq_q = qpool.tile([P, TC1], DT, tag="sqq",
                                          name="sq_q")
                        nc.scalar.square(out=sq_q, in_=psb_q)
                        qr = qpool.tile([P, TC1], DT, tag="ropeq", name="qr")
                        rope(psb_q, psw_q, cosq_sb[:, sl], sinq_sb[:, sl], qr)
                        q_ps.append((sq_q, qr))

                    # V directly in [t, d] layout: x block stationary
                    for i in range(TC1 // P):
                        ps_vt = ps_v.tile([P, P], F32, tag="vt", name="ps_vt")
                        for kt in range(NKT):
                            nc.tensor.matmul(
                                ps_vt, xts[:, kt, i * P:(i + 1) * P],
                                wv_sb[:, kt, :],
                                start=(kt == 0), stop=(kt == NKT - 1))
                        nc.vector.tensor_copy(
                            out=v_sb[:, tc_i * (TC1 // P) + i, :], in_=ps_vt)

                    # ---- norm reductions (PE, end of chunk) ----------------
                    # K: per key-block column sums of sq_k (sq stationary)
                    kb0 = tc_i * (TC1 // P)
                    ssqc = ps_sc.tile([P, TC1 // P], F32, tag="ssqc",
                                      name="ssqc")
                    for i in range(TC1 // P):
                        nc.tensor.matmul(ssqc[:, i:i + 1],
                                         sq_k[:, i * P:(i + 1) * P],
                                         ones_col, start=True, stop=True)
                    nc.scalar.activation(
                        out=rk_tiles[:, kb0:kb0 + TC1 // P], in_=ssqc,
                        func=AF.Sqrt, bias=eps_k[:], scale=float(1.0 / HD))
                    nc.vector.reciprocal(
                        out=rk_tiles[:, kb0:kb0 + TC1 // P],
                        in_=rk_tiles[:, kb0:kb0 + TC1 // P])
                    # Q: rq = 1/sqrt(ssq + HD*eps) applied to q columns
                    for h in range(G):
                        sq_q, qr = q_ps[h]
                        ssq = ps_sq.tile([1, TC1], F32, tag="ssq",
                                         name="ssq")
                        nc.tensor.matmul(ssq, ones_col, sq_q,
                                         start=True, stop=True)
                        sq_s = stri.tile([1, TC1], F32, tag="sqs",
                                         name="sq_s")
                        nc.scalar.activation(out=sq_s, in_=ssq, func=AF.Sqrt,
                                             bias=eps_q[:], scale=1.0)
                        rq = stri.tile([1, TC1], DT, tag="rq", name="rq")
                        nc.vector.reciprocal(out=rq, in_=sq_s)
                        rqB = tmpool.tile([P, TC1], DT, tag="rqB", name="rqB")
                        nc.gpsimd.partition_broadcast(rqB, rq)
                        nc.vector.tensor_mul(out=q_sb[h][:, sl], in0=qr,
                                             in1=rqB)

            # ---------------- Phase 2: attention + c_proj -------------------
            with tc.tile_pool(name="attn", bufs=4) as apool, \
                 tc.tile_pool(name="pb", bufs=6) as ppool, \
                 tc.tile_pool(name="pacc", bufs=4) as accpool, \
                 tc.tile_pool(name="ysb", bufs=4) as ypool, \
                 tc.tile_pool(name="p2st", bufs=3, space="PSUM") as ps_st, \
                 tc.tile_pool(name="p2ot", bufs=2, space="PSUM") as ps_ot, \
                 tc.tile_pool(name="p2rs", bufs=1, space="PSUM") as ps_rs, \
                 tc.tile_pool(name="p3ya", bufs=1, space="PSUM") as ps_ya, \
                 tc.tile_pool(name="p3yb", bufs=1, space="PSUM") as ps_yb:

                def cproj_steps(qc):
                    # 8 emission closures for q-chunk qc's 4 t-blocks
                    steps = []
                    for tb in range(4 * qc, 4 * qc + 4):
                        for jg in (0, 2):
                            def step(tb=tb, jg=jg):
                                ya = ps_ya.tile([P, TC], F32, tag="ya",
                                                name="ya")
                                yb = ps_yb.tile([P, TC], F32, tag="yb",
                                                name="yb")
                                for h in range(G):
                                    lhs = otn_sb[h][:, tb * P:(tb + 1) * P]
                                    nc.tensor.matmul(
                                        ya, lhs,
                                        wo_sb[:, h, jg * TC:(jg + 1) * TC],
                                        start=(h == 0), stop=(h == G - 1))
                                    nc.tensor.matmul(
                                        yb, lhs,
                                        wo_sb[:, h,
                                              (jg + 1) * TC:(jg + 2) * TC],
                                        start=(h == 0), stop=(h == G - 1))
                                for j, yp in ((jg, ya), (jg + 1, yb)):
                                    y_sb = ypool.tile([P, TC], DT, tag="y_sb",
                                                      name="y_sb")
                                    nc.vector.tensor_copy(out=y_sb, in_=yp)
                                    nc.sync.dma_start(
                                        out=y[tb * P:(tb + 1) * P,
                                              j * TC:(j + 1) * TC],
                                        in_=y_sb)
                            steps.append(step)
                    return steps

                for qc in range(NTC):
                    qsl = slice(qc * TC, (qc + 1) * TC)
                    nkb = 4 * (qc + 1)
                    steps = cproj_steps(qc - 1) if qc > 0 else []
                    # interleave points: 2 mid-loop per pair + pair ends
                    mids = {max(1, nkb // 3), max(2, (2 * nkb) // 3)}
                    for pair in ((0, 1), (2, 3)):
                        ot_ps = {}
                        acc = {}
                        for h in pair:
                            ot_ps[h] = ps_ot.tile([P, TC], F32, tag="ot",
                                                  name="ot_ps")
                            acc[h] = accpool.tile([P, TC], DT, tag="acc",
                                                  name="acc")
                        for kb in range(nkb):
                            r = kb - 4 * qc  # >=0 on diagonal blocks
                            c0 = max(r, 0) * P  # first valid q column
                            sts = {}
                            ps_ = {}
                            for h in pair:
                                st = ps_st.tile([P, TC], F32, tag="st",
                                                name="st_ps")
                                nc.tensor.matmul(
                                    st[:, c0:],
                                    kT_sb[:, kb * P:(kb + 1) * P],
                                    q_sb[h][:, qc * TC + c0:(qc + 1) * TC],
                                    start=True, stop=True)
                                sts[h] = st
                            for h in pair:
                                p_ = ppool.tile([P, TC], DT, tag="p",
                                                name="p_sb")
                                nc.scalar.activation(
                                    out=p_[:, c0:], in_=sts[h][:, c0:],
                                    func=AF.Exp, bias=expb[:],
                                    scale=rk_tiles[:, kb:kb + 1])
                                if r >= 0:
                                    # causal mask on the diagonal strip
                                    nc.gpsimd.affine_select(
                                        out=p_[:, c0:c0 + P],
                                        in_=p_[:, c0:c0 + P],
                                        pattern=[[1, P]],
                                        compare_op=ALU.is_ge,
                                        fill=0.0,
                                        base=0,
                                        channel_multiplier=-1)
                                if kb == 0:
                                    nc.vector.tensor_copy(out=acc[h],
                                                          in_=p_)
                                else:
                                    nc.vector.scalar_tensor_tensor(
                                        out=acc[h][:, c0:], in0=p_[:, c0:],
                                        scalar=1.0, in1=acc[h][:, c0:],
                                        op0=ALU.mult, op1=ALU.add)
                                ps_[h] = p_
                            for h in pair:
                                nc.tensor.matmul(
                                    ot_ps[h][:, c0:], v_sb[:, kb, :],
                                    ps_[h][:, c0:], start=(kb == 0),
                                    stop=(kb == nkb - 1))
                            if steps and (kb in mids):
                                steps.pop(0)()
                        for h in pair:
                            rs = ps_rs.tile([1, TC], F32, tag="rs",
                                            name="rs_ps")
                            nc.tensor.matmul(rs, ones_col, acc[h],
                                             start=True, stop=True)
                            recip = stri.tile([1, TC], DT, tag="recip",
                                              name="recip")
                            nc.vector.reciprocal(out=recip, in_=rs)
                            recipB = apool.tile([P, TC], DT, tag="recipB",
                                                name="recipB")
                            nc.gpsimd.partition_broadcast(recipB, recip)
                            nc.vector.tensor_mul(out=otn_sb[h][:, qsl],
                                                 in0=ot_ps[h], in1=recipB)
                            if steps:
                                steps.pop(0)()
                    while steps:
                        steps.pop(0)()
                # final chunk's c_proj
                for step in cproj_steps(NTC - 1):
                    step()

    nc.compile()
    return nc


_NC_CACHE = None


def _get_program():
    global _NC_CACHE
    if _NC_CACHE is None:
        _NC_CACHE = build_program()
    return _NC_CACHE


def _make_tables(pos, gamma2):
    half = HD // 2
    inv_freq = 1.0 / (THETA ** (np.arange(half, dtype=np.float64) / half))
    ang = (pos + np.arange(T, dtype=np.float64))[None, :] * inv_freq[:, None]
    cos = np.cos(ang)
    sin = np.sin(ang)
    cosq = np.concatenate([cos, cos], axis=0)
    sinq = np.concatenate([-sin, sin], axis=0)
    g2 = gamma2.astype(np.float64).reshape(P, 1)
    return (np.ascontiguousarray(cosq.astype(NP_DT)),
            np.ascontiguousarray(sinq.astype(NP_DT)),
            np.ascontiguousarray((cosq * g2).astype(NP_DT)),
            np.ascontiguousarray((sinq * g2).astype(NP_DT)))


def kernel(x, Wq, Wk, Wv, Wo, q_gamma, k_gamma, pos):
    x = np.asarray(x, dtype=np.float32)
    Wq = np.asarray(Wq, dtype=np.float32)
    Wk = np.asarray(Wk, dtype=np.float32)
    Wv = np.asarray(Wv, dtype=np.float32)
    Wo = np.asarray(Wo, dtype=np.float32)
    q_gamma = np.asarray(q_gamma, dtype=np.float32)
    k_gamma = np.asarray(k_gamma, dtype=np.float32)
    pos = int(np.asarray(pos))

    gamma2 = q_gamma * k_gamma
    cosq, sinq, cosk, sink = _make_tables(pos, gamma2)

    def st(a):
        return np.ascontiguousarray(a.astype(NP_DT))

    xTs = [st(x[b].T) for b in range(B)]
    in_maps = []
    for c in range(N_CORES):
        b, n = divmod(c, NKV)
        in_maps.append({
            "xT": xTs[b],
            "wq": st(Wq[:, n * G * HD:(n + 1) * G * HD]),
            "wk": st(Wk[:, n * HD:(n + 1) * HD]),
            "wv": st(Wv[:, n * HD:(n + 1) * HD]),
            "wo": st(Wo[n * G * HD:(n + 1) * G * HD, :]),
            "cosq": cosq,
            "sinq": sinq,
            "cosk": cosk,
            "sink": sink,
        })

    nc = _get_program()
    res = bass_utils.run_bass_kernel_spmd(nc, in_maps,
                                          core_ids=list(range(N_CORES)))
    out = np.zeros((B, T, D), dtype=np.float32)
    for c in range(N_CORES):
        b = c // NKV
        out[b] += res.results[c]["y"].astype(np.float32)
    return out


if __name__ == "__main__":
    build_program()
    print("program built OK")


# revision 16
# speedup vs baseline: 1.1364x; 1.0705x over previous
"""Trainium2 Bass kernel for nn_CausalSelfAttention_35931696398729.

Sharding: 8 cores = (batch b in {0,1}) x (kv-head n in {0..3}).
Each core computes its 4 query heads' causal GQA attention for its batch
plus the partial c_proj (rows of Wo for its heads); the host sums the 4
partials per batch.  No device collectives.

All matmul operands are fp16 (1 cyc/row in the cost model, like bf16,
with 8x lower quantization error).  PSUM stays f32.

Key structure:
 - qT/kT (d on partitions, t free) so scores come out as ST (keys on
   partitions, queries free) and PV consumes exp(ST) directly.
 - V is projected directly in [t, d] layout (x block as the stationary
   operand) -- no PE transposes.
 - QK RMSNorm: squares are taken from the PRE-RoPE psum (rotation
   preserves column norms).  q-side factor rq(t)/sqrt(HD) is multiplied
   into q during phase 1 (Pool partition_broadcast + DVE mul); k-side
   factor rk(s) rides the Exp activation's per-partition scale;
   gamma_q*gamma_k is folded into the K RoPE tables on the host.
 - softmax runs without max-subtraction but with a constant -2 bias in
   the exponent (softmax-invariant) so exp stays in fp16 range.
 - rowsum: P_acc += p on DVE (scalar_tensor_tensor, 4x mode), then one
   [1,TC] matmul per (head, q-chunk); 1/rowsum applied to OT columns.
 - c_proj for q-chunk qc-1 is interleaved into attention of qc at key-
   block granularity; y copies run on the Pool engine; y is fp16.
"""

import os
import sys

sys.path.insert(0, "/opt/trn_rl_repo")

import numpy as np

import concourse.bacc as bacc
import concourse.mybir as mybir
import concourse.tile as tile
from concourse import bass_utils

B, T, D = 2, 2048, 2048
NH, NKV, HD = 16, 4, 128
G = NH // NKV  # query heads per core
EPS = 1e-6
THETA = 10000.0
N_CORES = 8
P = 128
TC = 512            # q-chunk for attention / c_proj column chunk
NTC = T // TC       # 4
TC1 = 256           # t-chunk for phase-1 projections
NTC1 = T // TC1     # 8
NKT = D // P        # 16 contraction chunks
NTB = T // P        # 16 t-blocks
EXP_BIAS = -2.0     # constant exponent shift (softmax invariant)

F32 = mybir.dt.float32
DT = mybir.dt.float16
NP_DT = np.float16


def build_program():
    nc = bacc.Bacc("TRN2", target_bir_lowering=False, debug=False,
                   enable_asserts=False, num_devices=N_CORES)

    xT = nc.dram_tensor("xT", (D, T), DT, kind="ExternalInput").ap()
    wq = nc.dram_tensor("wq", (D, G * HD), DT, kind="ExternalInput").ap()
    wk = nc.dram_tensor("wk", (D, HD), DT, kind="ExternalInput").ap()
    wv = nc.dram_tensor("wv", (D, HD), DT, kind="ExternalInput").ap()
    wo = nc.dram_tensor("wo", (G * HD, D), DT, kind="ExternalInput").ap()
    cosq = nc.dram_tensor("cosq", (P, T), DT, kind="ExternalInput").ap()
    sinq = nc.dram_tensor("sinq", (P, T), DT, kind="ExternalInput").ap()
    cosk = nc.dram_tensor("cosk", (P, T), DT, kind="ExternalInput").ap()
    sink = nc.dram_tensor("sink", (P, T), DT, kind="ExternalInput").ap()
    y = nc.dram_tensor("y", (T, D), DT, kind="ExternalOutput").ap()

    AF = mybir.ActivationFunctionType
    ALU = mybir.AluOpType

    with tile.TileContext(nc) as tc, \
         nc.allow_low_precision(reason="fp16 matmul/softmax pipeline"):
        with tc.tile_pool(name="persist", bufs=1) as persist, \
             tc.tile_pool(name="stri", bufs=4) as stri:
            cosq_sb = persist.tile([P, T], DT)
            sinq_sb = persist.tile([P, T], DT)
            cosk_sb = persist.tile([P, T], DT)
            sink_sb = persist.tile([P, T], DT)
            ones_col = persist.tile([P, 1], DT)
            nc.vector.memset(ones_col, 1.0)
            warm_src = persist.tile([P, P], DT)
            nc.vector.memset(warm_src, 0.0)
            eps_k = persist.tile([P, 1], F32)
            nc.vector.memset(eps_k, EPS)
            eps_q = persist.tile([1, 1], F32)
            nc.vector.memset(eps_q, HD * EPS)
            expb = persist.tile([P, 1], F32)
            nc.vector.memset(expb, EXP_BIAS)

            q_sb = [persist.tile([P, T], DT, tag=f"q_sb{h}", name=f"q_sb{h}")
                    for h in range(G)]
            kT_sb = persist.tile([P, T], DT)
            v_sb = persist.tile([P, NTB, P], DT)
            rk_tiles = persist.tile([P, NTB], F32)
            wo_sb = persist.tile([P, G, D], DT)
            otn_sb = [persist.tile([P, T], DT, tag=f"otn{h}", name=f"otn{h}")
                      for h in range(G)]

            # PE warm-up: keep PE busy through the cold-clock ramp window
            # while the first x chunks stream in.
            with tc.tile_pool(name="warm", bufs=1, space="PSUM") as ps_w:
                warm_ps = ps_w.tile([1, P], F32)
                for _ in range(24):
                    nc.tensor.matmul(warm_ps, ones_col, warm_src,
                                     start=True, stop=True)

            # ---------------- Phase 1: projections + RoPE + norms -----------
            with tc.tile_pool(name="weights", bufs=1) as wpool, \
                 tc.tile_pool(name="xts", bufs=2) as xpool, \
                 tc.tile_pool(name="p1tmp", bufs=3) as tmpool, \
                 tc.tile_pool(name="p1q", bufs=5) as qpool, \
                 tc.tile_pool(name="p1ps", bufs=3, space="PSUM") as ps_a, \
                 tc.tile_pool(name="p1psv", bufs=2, space="PSUM") as ps_v, \
                 tc.tile_pool(name="p1sc", bufs=1, space="PSUM") as ps_sc, \
                 tc.tile_pool(name="p1sq", bufs=2, space="PSUM") as ps_sq:
                wq_sb = wpool.tile([P, NKT, G * HD], DT)
                wk_sb = wpool.tile([P, NKT, HD], DT)
                wv_sb = wpool.tile([P, NKT, HD], DT)
                nc.sync.dma_start(out=wk_sb,
                                  in_=wk.rearrange("(kt p) m -> p kt m", p=P))

                def swap_copy(psb, tag):
                    # halves-swapped copy (single-input ops may cross
                    # partition bases; two-input SB+SB ops may not)
                    psb_sw = tmpool.tile([P, TC1], DT, tag=tag, name=tag)
                    nc.vector.tensor_copy(out=psb_sw[0:64, :],
                                          in_=psb[64:128, :])
                    nc.vector.tensor_copy(out=psb_sw[64:128, :],
                                          in_=psb[0:64, :])
                    return psb_sw

                def rope(psb, psb_sw, cos_t, sin_t, dst):
                    # dst = psb * cos + swap(psb) * sin   (all fp16 SBUF,
                    # partition-aligned; sin table carries the sign fold)
                    tmp = tmpool.tile([P, TC1], DT, tag="ropetmp",
                                      name="ropetmp")
                    nc.vector.tensor_mul(out=tmp, in0=psb_sw, in1=sin_t)
                    tmp2 = tmpool.tile([P, TC1], DT, tag="ropetmp2",
                                       name="ropetmp2")
                    nc.vector.tensor_mul(out=tmp2, in0=psb, in1=cos_t)
                    nc.vector.tensor_add(out=dst, in0=tmp2, in1=tmp)

                for tc_i in range(NTC1):
                    sl = slice(tc_i * TC1, (tc_i + 1) * TC1)
                    xts = xpool.tile([P, NKT, TC1], DT, tag="xts", name="xts")
                    xre = xT[:, sl].rearrange("(kt p) m -> p kt m", p=P)
                    if tc_i == 0:
                        # stage startup loads: k weights + first x chunks
                        # first, then tables, then q/v weights, wo last
                        for kg in range(2):
                            nc.sync.dma_start(
                                out=xts[:, 4 * kg:4 * (kg + 1), :],
                                in_=xre[:, 4 * kg:4 * (kg + 1), :])
                        nc.sync.dma_start(out=cosk_sb, in_=cosk)
                        nc.sync.dma_start(out=sink_sb, in_=sink)
                        for kg in range(2, 4):
                            nc.sync.dma_start(
                                out=xts[:, 4 * kg:4 * (kg + 1), :],
                                in_=xre[:, 4 * kg:4 * (kg + 1), :])
                        nc.sync.dma_start(out=cosq_sb, in_=cosq)
                        nc.sync.dma_start(out=sinq_sb, in_=sinq)
                        wqre = wq.rearrange("(kt p) m -> p kt m", p=P)
                        for h in range(G):
                            nc.sync.dma_start(
                                out=wq_sb[:, :, h * HD:(h + 1) * HD],
                                in_=wqre[:, :, h * HD:(h + 1) * HD])
                        nc.sync.dma_start(
                            out=wv_sb,
                            in_=wv.rearrange("(kt p) m -> p kt m", p=P))
                        nc.sync.dma_start(
                            out=wo_sb,
                            in_=wo.rearrange("(h p) m -> p h m", p=P))
                    else:
                        for kg in range(4):
                            nc.sync.dma_start(
                                out=xts[:, 4 * kg:4 * (kg + 1), :],
                                in_=xre[:, 4 * kg:4 * (kg + 1), :])

                    # ---- PE: projections (K, Q heads, V) -------------------
                    ps_k = ps_a.tile([P, TC1], F32, tag="proj", name="ps_k")
                    for kt in range(NKT):
                        nc.tensor.matmul(ps_k, wk_sb[:, kt, :],
                                         xts[:, kt, :],
                                         start=(kt == 0), stop=(kt == NKT - 1))
                    # Act: psum -> fp16 SBUF copy + square (pre-RoPE norm)
                    psb_k = tmpool.tile([P, TC1], DT, tag="psb", name="psb_k")
                    nc.scalar.copy(out=psb_k, in_=ps_k)
                    psw_k = swap_copy(psb_k, "psw")
                    sq_k = tmpool.tile([P, TC1], DT, tag="sq", name="sq_k")
                    nc.scalar.square(out=sq_k, in_=psb_k)
                    rope(psb_k, psw_k, cosk_sb[:, sl], sink_sb[:, sl],
                         kT_sb[:, sl])

                    q_ps = []
                    for h in range(G):
                        ps_q = ps_a.tile([P, TC1], F32, tag="proj",
                                         name="ps_q")
                        for kt in range(NKT):
                            nc.tensor.matmul(
                                ps_q, wq_sb[:, kt, h * HD:(h + 1) * HD],
                                xts[:, kt, :],
                                start=(kt == 0), stop=(kt == NKT - 1))
                        psb_q = qpool.tile([P, TC1], DT, tag="psbq",
                                           name="psb_q")
                        nc.scalar.copy(out=psb_q, in_=ps_q)
                        psw_q = swap_copy(psb_q, "pswq")
                        sq_q = qpool.tile([P, TC1], DT, tag="sqq",
                                          name="sq_q")
                        nc.scalar.square(out=sq_q, in_=psb_q)
                        qr = qpool.tile([P, TC1], DT, tag="ropeq", name="qr")
                        rope(psb_q, psw_q, cosq_sb[:, sl], sinq_sb[:, sl], qr)
                        q_ps.append((sq_q, qr))

                    # V directly in [t, d] layout: x block stationary
                    for i in range(TC1 // P):
                        ps_vt = ps_v.tile([P, P], F32, tag="vt", name="ps_vt")
                        for kt in range(NKT):
                            nc.tensor.matmul(
                                ps_vt, xts[:, kt, i * P:(i + 1) * P],
                                wv_sb[:, kt, :],
                                start=(kt == 0), stop=(kt == NKT - 1))
                        nc.vector.tensor_copy(
                            out=v_sb[:, tc_i * (TC1 // P) + i, :], in_=ps_vt)

                    # ---- norm reductions (PE, end of chunk) ----------------
                    # K: per key-block column sums of sq_k (sq stationary)
                    kb0 = tc_i * (TC1 // P)
                    ssqc = ps_sc.tile([P, TC1 // P], F32, tag="ssqc",
                                      name="ssqc")
                    for i in range(TC1 // P):
                        nc.tensor.matmul(ssqc[:, i:i + 1],
                                         sq_k[:, i * P:(i + 1) * P],
                                         ones_col, start=True, stop=True)
                    nc.scalar.activation(
                        out=rk_tiles[:, kb0:kb0 + TC1 // P], in_=ssqc,
                        func=AF.Sqrt, bias=eps_k[:], scale=float(1.0 / HD))
                    nc.vector.reciprocal(
                        out=rk_tiles[:, kb0:kb0 + TC1 // P],
                        in_=rk_tiles[:, kb0:kb0 + TC1 // P])
                    # Q: rq = 1/sqrt(ssq + HD*eps) applied to q columns
                    for h in range(G):
                        sq_q, qr = q_ps[h]
                        ssq = ps_sq.tile([1, TC1], F32, tag="ssq",
                                         name="ssq")
                        nc.tensor.matmul(ssq, ones_col, sq_q,
                                         start=True, stop=True)
                        sq_s = stri.tile([1, TC1], F32, tag="sqs",
                                         name="sq_s")
                        nc.scalar.activation(out=sq_s, in_=ssq, func=AF.Sqrt,
                                             bias=eps_q[:], scale=1.0)
                        rq = stri.tile([1, TC1], DT, tag="rq", name="rq")
                        nc.vector.reciprocal(out=rq, in_=sq_s)
                        rqB = tmpool.tile([P, TC1], DT, tag="rqB", name="rqB")
                        nc.gpsimd.partition_broadcast(rqB, rq)
                        nc.vector.tensor_mul(out=q_sb[h][:, sl], in0=qr,
                                             in1=rqB)

            # ---------------- Phase 2: attention + c_proj -------------------
            with tc.tile_pool(name="attn", bufs=4) as apool, \
                 tc.tile_pool(name="pb", bufs=6) as ppool, \
                 tc.tile_pool(name="pacc", bufs=4) as accpool, \
                 tc.tile_pool(name="ysb", bufs=4) as ypool, \
                 tc.tile_pool(name="p2st", bufs=3, space="PSUM") as ps_st, \
                 tc.tile_pool(name="p2ot", bufs=2, space="PSUM") as ps_ot, \
                 tc.tile_pool(name="p2rs", bufs=1, space="PSUM") as ps_rs, \
                 tc.tile_pool(name="p3ya", bufs=1, space="PSUM") as ps_ya, \
                 tc.tile_pool(name="p3yb", bufs=1, space="PSUM") as ps_yb:

                def cproj_steps(qc):
                    # 8 emission closures for q-chunk qc's 4 t-blocks
                    steps = []
                    for tb in range(4 * qc, 4 * qc + 4):
                        for jg in (0, 2):
                            def step(tb=tb, jg=jg):
                                ya = ps_ya.tile([P, TC], F32, tag="ya",
                                                name="ya")
                                yb = ps_yb.tile([P, TC], F32, tag="yb",
                                                name="yb")
                                for h in range(G):
                                    lhs = otn_sb[h][:, tb * P:(tb + 1) * P]
                                    nc.tensor.matmul(
                                        ya, lhs,
                                        wo_sb[:, h, jg * TC:(jg + 1) * TC],
                                        start=(h == 0), stop=(h == G - 1))
                                    nc.tensor.matmul(
                                        yb, lhs,
                                        wo_sb[:, h,
                                              (jg + 1) * TC:(jg + 2) * TC],
                                        start=(h == 0), stop=(h == G - 1))
                                for j, yp in ((jg, ya), (jg + 1, yb)):
                                    y_sb = ypool.tile([P, TC], DT, tag="y_sb",
                                                      name="y_sb")
                                    if j % 2 == 0:
                                        nc.scalar.copy(out=y_sb, in_=yp)
                                    else:
                                        nc.vector.tensor_copy(out=y_sb,
                                                              in_=yp)
                                    nc.sync.dma_start(
                                        out=y[tb * P:(tb + 1) * P,
                                              j * TC:(j + 1) * TC],
                                        in_=y_sb)
                            steps.append(step)
                    return steps

                for qc in range(NTC):
                    qsl = slice(qc * TC, (qc + 1) * TC)
                    nkb = 4 * (qc + 1)
                    steps = cproj_steps(qc - 1) if qc > 0 else []
                    # interleave points: 2 mid-loop per pair + pair ends
                    mids = {max(1, nkb // 3), max(2, (2 * nkb) // 3)}
                    for pair in ((0, 1), (2, 3)):
                        ot_ps = {}
                        acc = {}
                        for h in pair:
                            ot_ps[h] = ps_ot.tile([P, TC], F32, tag="ot",
                                                  name="ot_ps")
                            acc[h] = accpool.tile([P, TC], DT, tag="acc",
                                                  name="acc")
                        for kb in range(nkb):
                            r = kb - 4 * qc  # >=0 on diagonal blocks
                            c0 = max(r, 0) * P  # first valid q column
                            sts = {}
                            ps_ = {}
                            for h in pair:
                                st = ps_st.tile([P, TC], F32, tag="st",
                                                name="st_ps")
                                nc.tensor.matmul(
                                    st[:, c0:],
                                    kT_sb[:, kb * P:(kb + 1) * P],
                                    q_sb[h][:, qc * TC + c0:(qc + 1) * TC],
                                    start=True, stop=True)
                                sts[h] = st
                            for h in pair:
                                p_ = ppool.tile([P, TC], DT, tag="p",
                                                name="p_sb")
                                nc.scalar.activation(
                                    out=p_[:, c0:], in_=sts[h][:, c0:],
                                    func=AF.Exp, bias=expb[:],
                                    scale=rk_tiles[:, kb:kb + 1])
                                if r >= 0:
                                    # causal mask on the diagonal strip
                                    nc.gpsimd.affine_select(
                                        out=p_[:, c0:c0 + P],
                                        in_=p_[:, c0:c0 + P],
                                        pattern=[[1, P]],
                                        compare_op=ALU.is_ge,
                                        fill=0.0,
                                        base=0,
                                        channel_multiplier=-1)
                                if kb == 0:
                                    nc.vector.tensor_copy(out=acc[h],
                                                          in_=p_)
                                else:
                                    nc.vector.tensor_add(
                                        out=acc[h][:, c0:],
                                        in0=acc[h][:, c0:],
                                        in1=p_[:, c0:])
                                ps_[h] = p_
                            for h in pair:
                                nc.tensor.matmul(
                                    ot_ps[h][:, c0:], v_sb[:, kb, :],
                                    ps_[h][:, c0:], start=(kb == 0),
                                    stop=(kb == nkb - 1))
                            if steps and (kb in mids):
                                steps.pop(0)()
                        for h in pair:
                            rs = ps_rs.tile([1, TC], F32, tag="rs",
                                            name="rs_ps")
                            nc.tensor.matmul(rs, ones_col, acc[h],
                                             start=True, stop=True)
                            recip = stri.tile([1, TC], DT, tag="recip",
                                              name="recip")
                            nc.vector.reciprocal(out=recip, in_=rs)
                            recipB = apool.tile([P, TC], DT, tag="recipB",
                                                name="recipB")
                            nc.gpsimd.partition_broadcast(recipB, recip)
                            nc.vector.tensor_mul(out=otn_sb[h][:, qsl],
                                                 in0=ot_ps[h], in1=recipB)
                            if steps:
                                steps.pop(0)()
                    while steps:
                        steps.pop(0)()
                # final chunk's c_proj
                for step in cproj_steps(NTC - 1):
                    step()

    nc.compile()
    return nc


_NC_CACHE = None


def _get_program():
    global _NC_CACHE
    if _NC_CACHE is None:
        _NC_CACHE = build_program()
    return _NC_CACHE


def _make_tables(pos, gamma2):
    half = HD // 2
    inv_freq = 1.0 / (THETA ** (np.arange(half, dtype=np.float64) / half))
    ang = (pos + np.arange(T, dtype=np.float64))[None, :] * inv_freq[:, None]
    cos = np.cos(ang)
    sin = np.sin(ang)
    cosq = np.concatenate([cos, cos], axis=0)
    sinq = np.concatenate([-sin, sin], axis=0)
    g2 = gamma2.astype(np.float64).reshape(P, 1)
    return (np.ascontiguousarray(cosq.astype(NP_DT)),
            np.ascontiguousarray(sinq.astype(NP_DT)),
            np.ascontiguousarray((cosq * g2).astype(NP_DT)),
            np.ascontiguousarray((sinq * g2).astype(NP_DT)))


def kernel(x, Wq, Wk, Wv, Wo, q_gamma, k_gamma, pos):
    x = np.asarray(x, dtype=np.float32)
    Wq = np.asarray(Wq, dtype=np.float32)
    Wk = np.asarray(Wk, dtype=np.float32)
    Wv = np.asarray(Wv, dtype=np.float32)
    Wo = np.asarray(Wo, dtype=np.float32)
    q_gamma = np.asarray(q_gamma, dtype=np.float32)
    k_gamma = np.asarray(k_gamma, dtype=np.float32)
    pos = int(np.asarray(pos))

    gamma2 = q_gamma * k_gamma
    cosq, sinq, cosk, sink = _make_tables(pos, gamma2)

    def st(a):
        return np.ascontiguousarray(a.astype(NP_DT))

    xTs = [st(x[b].T) for b in range(B)]
    in_maps = []
    for c in range(N_CORES):
        b, n = divmod(c, NKV)
        in_maps.append({
            "xT": xTs[b],
            "wq": st(Wq[:, n * G * HD:(n + 1) * G * HD]),
            "wk": st(Wk[:, n * HD:(n + 1) * HD]),
            "wv": st(Wv[:, n * HD:(n + 1) * HD]),
            "wo": st(Wo[n * G * HD:(n + 1) * G * HD, :]),
            "cosq": cosq,
            "sinq": sinq,
            "cosk": cosk,
            "sink": sink,
        })

    nc = _get_program()
    res = bass_utils.run_bass_kernel_spmd(nc, in_maps,
                                          core_ids=list(range(N_CORES)))
    out = np.zeros((B, T, D), dtype=np.float32)
    for c in range(N_CORES):
        b = c // NKV
        out[b] += res.results[c]["y"].astype(np.float32)
    return out


if __name__ == "__main__":
    build_program()
    print("program built OK")
